# revision 39
# baseline (speedup 1.0000x reference)
"""Fused multi-head attention kernel for Trainium2, SPMD over 8 NeuronCores.

Sharding: data-parallel over batch (B=8 -> 1 batch per core). No collectives.

Per-core algorithm (all shapes per core, b fixed; everything bf16 on PE —
fp8 was measured to blow the 2e-2 tolerance):
  x^T [E, L] and W_q/W_k/W_v host-transposed bf16; W_out bf16.
  Host precomputes expb^T[h, k, q] = exp(bias[h, q, k]) * (mask ? 0 : 1)
  in bf16, so the device never sees the mask and never adds the bias:
  softmax numerator is exp(S) * exp(bias) with masked entries exactly 0.
  Emission order overlaps projection with attention so the ACT exp stream
  (the per-head pacer) starts as early as possible:
    V-proj first (packed [L, H*65] with a ones column per head so the PV
    matmul also produces the softmax denominator), then Q(0)/K(0), then 16
    heads with the next projection woven into each head's kc loop as PE
    filler (head 2j -> Q(j+1), head 2j+1 -> K(j+1)).
  Attention per head h, per k-chunk (full q rows of S^T at a time):
    S^T[k,q] = K Q^T (bf16, contract=A=64, even/odd heads in disjoint PE
    row groups via base_partition).
    P^T = exp(S^T) on ACT (psum->sbuf, bf16, [128,1024] tiles).
    P'^T = P^T * expb^T on DVE (all-bf16 SBUF -> 2x mode; 2 of 8 on Pool).
    values^T[a,q] (+denominator row 64) = [V|1]^T-stationary matmul,
    lagged one kc so PE never waits on the exp/mult chain.
    Normalize: DVE reciprocal of denom row + per-partition scalar multiply
    into vnat [q, (kc, a)] bf16, then ONE SP dma_start_transpose per head
    writes vT[g] rows directly ([a, kc, q] = xbar transpose, 32 tiles *
    14ns) -- no PE transpose matmuls, no DVE staging copies.
  Head 14 weaves out-projection partials for lc=0 (ec 0..6) into its spare
  PE slots (the Q/K job queue is exhausted); head 15 has none left.
  Phase C: Y = values^T-stationary @ W_out^T, ec=7 (the last head pair)
  ordered last per accumulator so the final head's values never stall PE.
  Stores split Pool/SP, the last one in quarters to shorten the drain.
  DMA engine split (transfer time serializes on the issuing engine):
  Pool: xT, wk, y(even lc); SP: wv, wq, wo, expb, vT transposes, y(odd).
"""

import sys

sys.path.insert(0, "/opt/trn_rl_repo")

import numpy as np
from contextlib import ExitStack

B, L, E, H, A = 8, 1024, 1024, 16, 64
SCALE = float(A) ** -0.5
KT = L // 128  # 8 k-chunks of 128

_cache = {}


def _build_nc():
    import concourse.bass as bass
    import concourse.bacc as bacc
    import concourse.tile as tile
    from concourse import mybir

    f32 = mybir.dt.float32
    bf16 = mybir.dt.bfloat16
    PSUM = bass.MemorySpace.PSUM
    Exp = mybir.ActivationFunctionType.Exp

    nc = bacc.Bacc(None, target_bir_lowering=False)
    xT_d = nc.dram_tensor("xT", [E, L], bf16, kind="ExternalInput")
    wq_d = nc.dram_tensor("wq", [E, E], bf16, kind="ExternalInput")
    wk_d = nc.dram_tensor("wk", [E, E], bf16, kind="ExternalInput")
    wv_d = nc.dram_tensor("wv", [E, H * 65], bf16, kind="ExternalInput")
    wo_d = nc.dram_tensor("wo", [E, E], bf16, kind="ExternalInput")
    expb_d = nc.dram_tensor("expb", [H, L, L], bf16, kind="ExternalInput")
    y_d = nc.dram_tensor("y", [L, E], f32, kind="ExternalOutput")

    with nc.allow_low_precision(reason="bf16 attention; tolerance 2e-2"), \
         tile.TileContext(nc) as tc, ExitStack() as top:
        pp = top.enter_context(tc.tile_pool(name="persist", bufs=8))

        qT = [pp.tile([128, L], bf16, tag="qT", name=f"qT{_}") for _ in range(8)]
        kTt = [pp.tile([128, L], bf16, tag="kT", name=f"kT{_}") for _ in range(8)]
        vs = [pp.tile([128, H * 65], bf16, tag="vs", name=f"vs{_}") for _ in range(8)]
        vT = [pp.tile([128, L], bf16, tag="vT", name=f"vT{_}") for _ in range(8)]

        with tc.tile_pool(name="m_eb", bufs=4) as ebp, \
             tc.tile_pool(name="m_w", bufs=4) as wp, \
             tc.tile_pool(name="m_wk", bufs=2) as wkp, \
             tc.tile_pool(name="m_x", bufs=2) as xp, \
             tc.tile_pool(name="m_wo", bufs=8) as wop:
            # input DMAs: xT on Pool; wv, wq, wo on SP; wk on Pool after xT.
            # First slivers of x and wv are split out so the very first V
            # matmul (needs x cols 0:128, wv cols 0:512) unblocks early.
            xs4 = [xp.tile([128, 4, L], bf16, tag="xs", name=f"xs{_}") for _ in range(2)]
            nc.gpsimd.dma_start(xs4[0][:, 0, 0:128], xT_d[0:128, 0:128])
            nc.gpsimd.dma_start(xs4[0][:, 0, 128:L], xT_d[0:128, 128:L])
            for t in range(2):
                nq = 4 if t == 0 else 2
                for hh in range(nq):
                    if t == 0 and hh == 0:
                        continue
                    w_ = 4 // nq
                    nc.gpsimd.dma_start(
                        xs4[t][:, hh * w_:(hh + 1) * w_, :],
                        xT_d[t * 512 + hh * w_ * 128:
                             t * 512 + (hh + 1) * w_ * 128, :]
                        .rearrange("(t p) e -> p t e", p=128))

            def load_w(w_d, nm, pool, eng, fine=False, head_cols=0):
                wt = [pool.tile([128, 4, w_d.shape[1]], bf16, tag="wt",
                                name=f"{nm}{_}") for _ in range(2)]
                if head_cols:
                    eng.dma_start(wt[0][:, 0, 0:head_cols],
                                  w_d[0:128, 0:head_cols])
                    eng.dma_start(wt[0][:, 0, head_cols:],
                                  w_d[0:128, head_cols:])
                for t in range(2):
                    nq = 4 if (fine and t == 0) else 2
                    w_ = 4 // nq
                    for hh in range(nq):
                        if head_cols and t == 0 and hh == 0 and w_ == 1:
                            continue
                        eng.dma_start(
                            wt[t][:, hh * w_:(hh + 1) * w_, :],
                            w_d[t * 512 + hh * w_ * 128:
                                t * 512 + (hh + 1) * w_ * 128, :]
                            .rearrange("(t p) e -> p t e", p=128))
                return wt

            wtv = load_w(wv_d, "wtv", wp, nc.sync, fine=True, head_cols=520)
            wtq = load_w(wq_d, "wtq", wp, nc.sync)
            wtk = load_w(wk_d, "wtk", wkp, nc.gpsimd)
            wot = [wop.tile([128, E], bf16, tag="wo", name=f"wo{_}") for _ in range(8)]
            for t in range(8):
                nc.sync.dma_start(wot[t][:], wo_d[t * 128:(t + 1) * 128, :])

            ebs = {}

            def issue_eb(h):
                # two half-tiles per head: the 4-slot ring lets half 0's DMA
                # start once head h-2 is past kc3 (instead of fully done),
                # and halves the quantum behind which SP-queue neighbours
                # (the vT pair transposes) can get stuck
                eb_lo = ebp.tile([128, KT // 2, L], bf16, tag="eb",
                                 name=f"eb{h}lo")
                eb_hi = ebp.tile([128, KT // 2, L], bf16, tag="eb",
                                 name=f"eb{h}hi")
                for t, ebt in ((0, eb_lo), (1, eb_hi)):
                    nc.sync.dma_start(
                        ebt[:],
                        expb_d[h, t * 512:(t + 1) * 512, :]
                        .rearrange("(kt p) q -> p kt q", p=128))
                ebs[h] = (eb_lo, eb_hi)

            for h in range(3):
                issue_eb(h)

            def xsl(k):
                return xs4[k // 4][:, k % 4, :]

            # ---------------- V projection ----------------
            # Q(0)/K(0) ride in the same psum pool right after V so the
            # first head's S matmuls aren't blocked on a cross-pool handoff.
            segs = [(0, 512), (512, 512), (1024, 16)]
            with tc.tile_pool(name="v_ps", bufs=2, space=PSUM) as vps, \
                 tc.tile_pool(name="v_pro", bufs=1, space=PSUM) as vpro:

                def emit_prologue():
                    for wts, dst, ceng in ((wtq, qT, "v"), (wtk, kTt, "s")):
                        ps = vpro.tile([128, L], f32, tag="pspro", name="ps_pro")
                        for k in range(8):
                            for lh in range(2):
                                nc.tensor.matmul(
                                    ps[:, lh * 512:(lh + 1) * 512],
                                    wts[k // 4][:, k % 4, 0:128],
                                    xsl(k)[:, lh * 512:(lh + 1) * 512],
                                    start=(k == 0), stop=(k == 7))
                        if ceng == "v":
                            nc.vector.tensor_copy(dst[0][:], ps[:])
                        else:
                            nc.scalar.copy(dst[0][:], ps[:])

                for lc in range(8):
                    psv = vps.tile([128, H * 65], f32, tag="psv")
                    for k in range(8):
                        for off, n in segs:
                            nc.tensor.matmul(
                                psv[:, off:off + n],
                                xsl(k)[:, lc * 128:(lc + 1) * 128],
                                wtv[k // 4][:, k % 4, off:off + n],
                                start=(k == 0), stop=(k == 7))
                    nc.scalar.copy(vs[lc][:, 0:520], psv[:, 0:520])
                    nc.vector.tensor_copy(vs[lc][:, 520:], psv[:, 520:])
                    # ones column per head (col 64 of each 65-wide slot)
                    ones_cols = vs[lc][:].rearrange(
                        "p (h c) -> p h c", c=65)[:, :, 64:65]
                    nc.vector.memset(ones_cols, 1.0)
                    if lc == 5:
                        # prologue Q(0)/K(0) here: its copies jump ahead of
                        # the remaining V copies in the ACT/DVE queues
                        emit_prologue()

            # ---------- heads with woven projections ----------
            with tc.tile_pool(name="b_st", bufs=2, space=PSUM) as stp, \
                 tc.tile_pool(name="b_pvn", bufs=1, space=PSUM) as pvnp, \
                 tc.tile_pool(name="b_pj", bufs=2, space=PSUM) as pjp, \
                 tc.tile_pool(name="b_pt", bufs=3) as ptp, \
                 tc.tile_pool(name="b_pm", bufs=3) as pmp, \
                 tc.tile_pool(name="b_nrm", bufs=2) as nrm:

                def emit_proj(ps_q, wts, m, k, g):
                    nc.tensor.matmul(
                        ps_q[:],
                        wts[k // 4][:, k % 4, m * 128:(m + 1) * 128],
                        xsl(k)[:, g * 256:(g + 1) * 256],
                        start=(k == 0), stop=(k == 7))

                def emit_s(h, kc, st):
                    g, hb = h // 2, (h % 2) * 64
                    for qh in range(2):
                        nc.tensor.matmul(
                            st[:, qh * 512:(qh + 1) * 512],
                            kTt[g][hb:hb + 64, kc * 128:(kc + 1) * 128],
                            qT[g][hb:hb + 64, qh * 512:(qh + 1) * 512],
                            start=True, stop=True)

                pending_free = [None]
                hoisted = [None]
                vnat2 = [None]
                # lc=0 out-projection partials woven into head 14 (whose Q/K
                # job queue is empty): [eh][ec<=6 done] psum halves
                psy0 = [None, None]
                # proj jobs are 32 quarter-matmuls (4 psum quarter-groups of
                # 8 contract steps, copy inline after each group), consumed
                # 4 per kc: PE's time between consecutive S(kc) completions
                # stays ~1.07us, just above ACT's 1.04us exp cadence, so the
                # exp stream never waits on a batch of woven matmuls (any
                # ACT slip compounds across heads and stalls PE via the
                # S-tile ring WAR).
                weave_per_kc = [4] * 8
                psy0_weave = [2, 2, 2, 2, 2, 2, 0, 0]
                psy0b_weave = [1, 1, 0, 0, 0, 0, 0, 0]

                def attn_head(h, job):
                    g, hb = h // 2, (h % 2) * 64
                    eb = ebs.pop(h)
                    if job == "psy0":
                        # lc0 out-proj partials, ec 0..5 only: vT[0..5] were
                        # delivered many heads ago, so no unit can ever park
                        # in PE's in-order queue ahead of this head's S
                        # matmuls (which pace the exp stream's finish)
                        psy0[0] = pjp.tile([128, 512], f32, tag="pj",
                                           name="psy0h0")
                        psy0[1] = pjp.tile([128, 512], f32, tag="pj",
                                           name="psy0h1")
                        jobs = [(eh, ec) for ec in range(6)
                                for eh in range(2)]
                    elif job == "psy0b":
                        # head 15: add ec6 (vT[6]'s transpose landed before
                        # this head began)
                        jobs = [(0, 6), (1, 6)]
                    elif job is not None:
                        wts, dst, m = job
                        phq = [None] * 4
                        jobs = [(k, g) for g in range(4) for k in range(8)]
                    else:
                        jobs = []
                    pvn = pvnp.tile([128, KT, 128], f32, tag="pvn")
                    pmq = []
                    ji = 0
                    if hoisted[0] is not None:
                        st_cur = hoisted[0]
                        hoisted[0] = None
                    else:
                        st_cur = stp.tile([128, L], f32, tag="st")
                        emit_s(h, 0, st_cur)
                    for kc in range(KT):
                        # PV first: its pm is ready (multiplied ~1.5 kc ago),
                        # so these 8 matmuls never park in the dispatch
                        # window; a parked batch would flood the 4-deep
                        # bypass and stall everything woven behind it
                        if len(pmq) == 2:
                            pkc = kc - 2
                            ppm = pmq.pop(0)
                            for qc in range(8):
                                nc.tensor.matmul(
                                    pvn[:, qc, 0:65],
                                    ppm[:, qc * 128:(qc + 1) * 128],
                                    vs[pkc][:, h * 65:(h + 1) * 65],
                                    start=(pkc == 0 and qc % 4 == 0),
                                    stop=False,
                                    skip_group_check=True)
                        st_next = None
                        if kc + 1 < KT:
                            st_next = stp.tile([128, L], f32, tag="st")
                            emit_s(h, kc + 1, st_next)
                        pt = ptp.tile([128, L], bf16, tag="pt")
                        nc.scalar.activation(pt[:], st_cur[:], Exp,
                                             scale=SCALE)
                        pm = pmp.tile([128, L], bf16, tag="pm")
                        # kc7's multiply on Pool keeps DVE clear for the
                        # final weave copy, which the hoisted S depends on
                        pool_mul = kc in (2, 5) or (kc == 7 and h < H - 1)
                        meng = nc.gpsimd if pool_mul else nc.vector
                        meng.tensor_mul(pm[:], pt[:], eb[kc // 4][:, kc % 4, :])
                        if kc == 1 and h + 3 < H:
                            # eb issued at kc1: its DMA then queues on SP
                            # BEHIND this head's vT transpose (emitted in
                            # pending_free at kc0), so a WAR-blocked eb
                            # can't head-of-line-block the transpose
                            issue_eb(h + 3)
                        if job == "psy0":
                            wkc = psy0_weave
                        elif job == "psy0b":
                            wkc = psy0b_weave
                        else:
                            wkc = weave_per_kc
                        for _ in range(wkc[kc]):
                            if ji < len(jobs):
                                if job in ("psy0", "psy0b"):
                                    eh, ec = jobs[ji]
                                    nc.tensor.matmul(
                                        psy0[eh][:],
                                        vT[ec][:, 0:128],
                                        wot[ec][:, eh * 512:(eh + 1) * 512],
                                        start=(ec == 0), stop=False)
                                else:
                                    k_, g_ = jobs[ji]
                                    if k_ == 0:
                                        phq[g_] = pjp.tile(
                                            [128, 256], f32, tag="pj",
                                            name=f"ph{m}g{g_}")
                                    emit_proj(phq[g_], wts, m, k_, g_)
                                    if k_ == 7:
                                        nc.vector.tensor_copy(
                                            dst[m][:, g_ * 256:(g_ + 1) * 256],
                                            phq[g_][:])
                                ji += 1
                        if kc == KT - 1 and h + 1 < H:
                            # hoisted next-head S emitted last: the weave's
                            # final qT/kT copy it reads has landed by then
                            nst = stp.tile([128, L], f32, tag="st")
                            emit_s(h + 1, 0, nst)
                            hoisted[0] = nst
                        pmq.append(pm)
                        st_cur = st_next
                        if kc == 0 and pending_free[0] is not None:
                            pending_free[0]()
                            pending_free[0] = None
                    for pkc in (KT - 2, KT - 1):
                        ppm = pmq.pop(0)
                        for qc in range(8):
                            nc.tensor.matmul(
                                pvn[:, qc, 0:65],
                                ppm[:, qc * 128:(qc + 1) * 128],
                                vs[pkc][:, h * 65:(h + 1) * 65],
                                start=False, stop=(pkc == KT - 1),
                                skip_group_check=True)
                    # normalize (values-natural: denominator is per-partition)
                    # into the pair staging tile [q, (kt, ha)]; after the odd
                    # head, one xbar DMA transpose writes vT[g] = [ha, (kt, q)]
                    # (out partition span must equal the 128-wide free blocks
                    # for the hardware xbar lowering, hence pair granularity).
                    r8 = nrm.tile([128, KT, 1], f32, tag="stage")
                    if h % 2 == 0:
                        vnat2[0] = nrm.tile([128, KT, 128], bf16, tag="vtmp",
                                            name=f"vnat{g}")
                    vn = vnat2[0]

                    def free_pv():
                        nc.vector.reciprocal(r8[:], pvn[:, :, 64:65])
                        for qc in range(8):
                            nc.vector.tensor_scalar_mul(
                                vn[:, qc, hb:hb + 64], pvn[:, qc, 0:64],
                                r8[:, qc, :])
                        if h % 2 == 1:
                            nc.sync.dma_start_transpose(
                                vT[g][:].rearrange("p (kt q) -> p kt q", q=128),
                                vn[:].rearrange("p kt a -> p (kt a)"))

                    pending_free[0] = free_pv

                for h in range(H):
                    j = h // 2
                    if h % 2 == 0:
                        job = (wtq, qT, j + 1) if j + 1 < 8 else "psy0"
                    else:
                        job = (wtk, kTt, j + 1) if j + 1 < 8 else "psy0b"
                    attn_head(h, job)
                pending_free[0]()

                # ------------- Phase C: output projection -------------
                # ec=7 (head pair 7, finalized just above) is ordered last
                # per accumulator so its transpose DMA never stalls PE.
                with tc.tile_pool(name="c_y", bufs=2) as yp, \
                     tc.tile_pool(name="c_yq", bufs=4) as yqp:

                    def psy_mm(psy, pcol0, lc, eh, ec, start, stop):
                        nc.tensor.matmul(
                            psy[:, pcol0:pcol0 + 512],
                            vT[ec][:, lc * 128:(lc + 1) * 128],
                            wot[ec][:, eh * 512:(eh + 1) * 512],
                            start=start, stop=stop)

                    def store_y(lc, ysrc, split):
                        # ysrc: list of (psum_tile, col0, eh); copies
                        # Pool/DVE, DMA Pool/SP
                        y = yp.tile([128, E], f32, tag="y")
                        for pst, col0, eh in ysrc:
                            if split:
                                for q in range(2):
                                    cols = slice(eh * 512 + q * 256,
                                                 eh * 512 + (q + 1) * 256)
                                    pcols = slice(col0 + q * 256,
                                                  col0 + (q + 1) * 256)
                                    if (eh + q) % 2:
                                        nc.scalar.copy(y[:, cols],
                                                       pst[:, pcols])
                                        nc.gpsimd.dma_start(
                                            y_d[lc * 128:(lc + 1) * 128, cols],
                                            y[:, cols])
                                    else:
                                        nc.vector.tensor_copy(y[:, cols],
                                                              pst[:, pcols])
                                        nc.sync.dma_start(
                                            y_d[lc * 128:(lc + 1) * 128, cols],
                                            y[:, cols])
                            else:
                                if eh:
                                    nc.scalar.copy(y[:, 512:E],
                                                   pst[:, col0:col0 + 512])
                                    nc.sync.dma_start(
                                        y_d[lc * 128:(lc + 1) * 128, 512:E],
                                        y[:, 512:E])
                                else:
                                    nc.vector.tensor_copy(
                                        y[:, 0:512], pst[:, col0:col0 + 512])
                                    nc.gpsimd.dma_start(
                                        y_d[lc * 128:(lc + 1) * 128, 0:512],
                                        y[:, 0:512])

                    # lc1, lc2 from the freed S pool: ec0..6 first (vT[7]'s
                    # transpose is still in flight)
                    psyA = stp.tile([128, L], f32, tag="st", name="psyA")
                    psyB = stp.tile([128, L], f32, tag="st", name="psyB")
                    for ec in range(7):
                        for eh in range(2):
                            psy_mm(psyA, eh * 512, 1, eh, ec,
                                   start=(ec == 0), stop=False)
                    for ec in range(7):
                        for eh in range(2):
                            psy_mm(psyB, eh * 512, 2, eh, ec,
                                   start=(ec == 0), stop=False)
                    # vT[7] has landed by now
                    for eh in range(2):
                        psy_mm(psy0[eh], 0, 0, eh, 7, start=False, stop=True)
                    store_y(0, [(psy0[0], 0, 0), (psy0[1], 0, 1)], split=False)
                    for eh in range(2):
                        psy_mm(psyA, eh * 512, 1, eh, 7, start=False, stop=True)
                    store_y(1, [(psyA, 0, 0), (psyA, 512, 1)], split=False)
                    for eh in range(2):
                        psy_mm(psyB, eh * 512, 2, eh, 7, start=False, stop=True)
                    store_y(2, [(psyB, 0, 0), (psyB, 512, 1)], split=False)
                    for lc in range(3, 7):
                        psy = stp.tile([128, L], f32, tag="st", name=f"psy{lc}")
                        for ec in range(8):
                            for eh in range(2):
                                psy_mm(psy, eh * 512, lc, eh, ec,
                                       start=(ec == 0), stop=(ec == 7))
                        store_y(lc, [(psy, 0, 0), (psy, 512, 1)], split=False)
                    # last chunk: accumulate per column-quarter in SEPARATE
                    # psum tiles (per-tile dep tracking would serialize a
                    # shared tile against each quarter's copy) so each
                    # quarter's copy+DMA pipelines behind the next quarter's
                    # matmuls and the final DMA is small and issued early
                    for q in range(4):
                        pool = stp if q < 2 else pjp
                        psq = pool.tile([128, 256], f32,
                                        tag="st" if q < 2 else "pj",
                                        name=f"psq{q}")
                        for ec in range(8):
                            nc.tensor.matmul(
                                psq[:],
                                vT[ec][:, 7 * 128:8 * 128],
                                wot[ec][:, q * 256:(q + 1) * 256],
                                start=(ec == 0), stop=(ec == 7))
                        y = yqp.tile([128, 256], f32, tag="yq", name=f"yq{q}")
                        if q == 3:
                            # final piece split in two so the very last DMA
                            # is small and issued as early as possible
                            for e in range(2):
                                cols = slice(q * 256 + e * 128,
                                             q * 256 + (e + 1) * 128)
                                ecols = slice(e * 128, (e + 1) * 128)
                                if e:
                                    nc.scalar.copy(y[:, ecols], psq[:, ecols])
                                    nc.sync.dma_start(y_d[7 * 128:L, cols],
                                                      y[:, ecols])
                                else:
                                    nc.vector.tensor_copy(y[:, ecols],
                                                          psq[:, ecols])
                                    nc.gpsimd.dma_start(y_d[7 * 128:L, cols],
                                                        y[:, ecols])
                        else:
                            cols = slice(q * 256, (q + 1) * 256)
                            if q % 2:
                                nc.scalar.copy(y[:], psq[:])
                                nc.sync.dma_start(y_d[7 * 128:L, cols], y[:])
                            else:
                                nc.vector.tensor_copy(y[:], psq[:])
                                nc.gpsimd.dma_start(y_d[7 * 128:L, cols], y[:])

    nc.finalize()
    return nc


def _prep_host(inputs):
    import ml_dtypes

    bf = ml_dtypes.bfloat16
    emb = np.asarray(inputs["embeddings"], np.float32)
    mask = np.asarray(inputs["attn_mask"])
    bias = np.asarray(inputs["attn_bias"], np.float32)
    Wqkv = np.asarray(inputs["W_qkv"], np.float32)
    Wout = np.asarray(inputs["W_out"], np.float32)

    Wr = Wqkv.reshape(H, 3 * A, E)
    WqT = np.ascontiguousarray(Wr[:, 0:A, :].reshape(E, E).T.astype(bf))
    WkT = np.ascontiguousarray(Wr[:, A:2 * A, :].reshape(E, E).T.astype(bf))
    Wv_T = Wr[:, 2 * A:3 * A, :].reshape(E, E).T  # [e, (h,a)]
    WvT = np.zeros((E, H * 65), np.float32)
    for h in range(H):
        WvT[:, h * 65:h * 65 + 64] = Wv_T[:, h * 64:(h + 1) * 64]
    WvT = np.ascontiguousarray(WvT.astype(bf))
    WoT = np.ascontiguousarray(Wout.T.astype(bf))

    if mask.dtype != np.bool_:
        mask = mask != 0

    in_maps = []
    for b in range(B):
        # expb^T[h, k, q] = exp(bias[b, h, q, k]) masked to 0, bf16
        expb = np.where(mask[b], 0.0, np.exp(bias[b]))  # [H, q, k]
        expbT = np.ascontiguousarray(expb.transpose(0, 2, 1).astype(bf))
        in_maps.append({
            "xT": np.ascontiguousarray(emb[b].T.astype(bf)),
            "wq": WqT, "wk": WkT, "wv": WvT, "wo": WoT,
            "expb": expbT,
        })
    return in_maps


def _run(inputs, trace=False):
    from concourse.bass_utils import run_bass_kernel_spmd

    if "nc" not in _cache:
        _cache["nc"] = _build_nc()
    nc = _cache["nc"]
    in_maps = _prep_host(inputs)
    res = run_bass_kernel_spmd(nc, in_maps, core_ids=list(range(8)), trace=trace)
    out = np.stack([np.asarray(res.results[c]["y"], np.float32) for c in range(B)], axis=0)
    return out, res


def kernel(**inputs) -> np.ndarray:
    out, _ = _run(inputs, trace=False)
    return out


def kernel_traced(**inputs):
    return _run(inputs, trace=True)


# revision 40
# speedup vs baseline: 1.0012x; 1.0012x over previous
"""Fused multi-head attention kernel for Trainium2, SPMD over 8 NeuronCores.

Sharding: data-parallel over batch (B=8 -> 1 batch per core). No collectives.

Per-core algorithm (all shapes per core, b fixed; everything bf16 on PE —
fp8 was measured to blow the 2e-2 tolerance):
  x^T [E, L] and W_q/W_k/W_v host-transposed bf16; W_out bf16.
  Host precomputes expb^T[h, k, q] = exp(bias[h, q, k]) * (mask ? 0 : 1)
  in bf16, so the device never sees the mask and never adds the bias:
  softmax numerator is exp(S) * exp(bias) with masked entries exactly 0.
  Emission order overlaps projection with attention so the ACT exp stream
  (the per-head pacer) starts as early as possible:
    V-proj first (packed [L, H*65] with a ones column per head so the PV
    matmul also produces the softmax denominator), then Q(0)/K(0), then 16
    heads with the next projection woven into each head's kc loop as PE
    filler (head 2j -> Q(j+1), head 2j+1 -> K(j+1)).
  Attention per head h, per k-chunk (full q rows of S^T at a time):
    S^T[k,q] = K Q^T (bf16, contract=A=64, even/odd heads in disjoint PE
    row groups via base_partition).
    P^T = exp(S^T) on ACT (psum->sbuf, bf16, [128,1024] tiles).
    P'^T = P^T * expb^T on DVE (all-bf16 SBUF -> 2x mode; 2 of 8 on Pool).
    values^T[a,q] (+denominator row 64) = [V|1]^T-stationary matmul,
    lagged one kc so PE never waits on the exp/mult chain.
    Normalize: DVE reciprocal of denom row + per-partition scalar multiply
    into vnat [q, (kc, a)] bf16, then ONE SP dma_start_transpose per head
    writes vT[g] rows directly ([a, kc, q] = xbar transpose, 32 tiles *
    14ns) -- no PE transpose matmuls, no DVE staging copies.
  Head 14 weaves out-projection partials for lc=0 (ec 0..6) into its spare
  PE slots (the Q/K job queue is exhausted); head 15 has none left.
  Phase C: Y = values^T-stationary @ W_out^T, ec=7 (the last head pair)
  ordered last per accumulator so the final head's values never stall PE.
  Stores split Pool/SP, the last one in quarters to shorten the drain.
  DMA engine split (transfer time serializes on the issuing engine):
  Pool: xT, wk, y(even lc); SP: wv, wq, wo, expb, vT transposes, y(odd).
"""

import sys

sys.path.insert(0, "/opt/trn_rl_repo")

import numpy as np
from contextlib import ExitStack

B, L, E, H, A = 8, 1024, 1024, 16, 64
SCALE = float(A) ** -0.5
KT = L // 128  # 8 k-chunks of 128

_cache = {}


def _build_nc():
    import concourse.bass as bass
    import concourse.bacc as bacc
    import concourse.tile as tile
    from concourse import mybir

    f32 = mybir.dt.float32
    bf16 = mybir.dt.bfloat16
    PSUM = bass.MemorySpace.PSUM
    Exp = mybir.ActivationFunctionType.Exp

    nc = bacc.Bacc(None, target_bir_lowering=False)
    xT_d = nc.dram_tensor("xT", [E, L], bf16, kind="ExternalInput")
    wq_d = nc.dram_tensor("wq", [E, E], bf16, kind="ExternalInput")
    wk_d = nc.dram_tensor("wk", [E, E], bf16, kind="ExternalInput")
    wv_d = nc.dram_tensor("wv", [E, H * 65], bf16, kind="ExternalInput")
    wo_d = nc.dram_tensor("wo", [E, E], bf16, kind="ExternalInput")
    expb_d = nc.dram_tensor("expb", [H, L, L], bf16, kind="ExternalInput")
    y_d = nc.dram_tensor("y", [L, E], f32, kind="ExternalOutput")

    with nc.allow_low_precision(reason="bf16 attention; tolerance 2e-2"), \
         tile.TileContext(nc) as tc, ExitStack() as top:
        pp = top.enter_context(tc.tile_pool(name="persist", bufs=8))

        qT = [pp.tile([128, L], bf16, tag="qT", name=f"qT{_}") for _ in range(8)]
        kTt = [pp.tile([128, L], bf16, tag="kT", name=f"kT{_}") for _ in range(8)]
        vs = [pp.tile([128, H * 65], bf16, tag="vs", name=f"vs{_}") for _ in range(8)]
        vT = [pp.tile([128, L], bf16, tag="vT", name=f"vT{_}") for _ in range(8)]

        with tc.tile_pool(name="m_eb", bufs=4) as ebp, \
             tc.tile_pool(name="m_w", bufs=4) as wp, \
             tc.tile_pool(name="m_wk", bufs=2) as wkp, \
             tc.tile_pool(name="m_x", bufs=2) as xp, \
             tc.tile_pool(name="m_wo", bufs=8) as wop:
            # input DMAs: xT on Pool; wv, wq, wo on SP; wk on Pool after xT.
            # First slivers of x and wv are split out so the very first V
            # matmul (needs x cols 0:128, wv cols 0:512) unblocks early.
            xs4 = [xp.tile([128, 4, L], bf16, tag="xs", name=f"xs{_}") for _ in range(2)]
            nc.gpsimd.dma_start(xs4[0][:, 0, 0:128], xT_d[0:128, 0:128])
            nc.gpsimd.dma_start(xs4[0][:, 0, 128:L], xT_d[0:128, 128:L])
            for t in range(2):
                nq = 4 if t == 0 else 2
                for hh in range(nq):
                    if t == 0 and hh == 0:
                        continue
                    w_ = 4 // nq
                    nc.gpsimd.dma_start(
                        xs4[t][:, hh * w_:(hh + 1) * w_, :],
                        xT_d[t * 512 + hh * w_ * 128:
                             t * 512 + (hh + 1) * w_ * 128, :]
                        .rearrange("(t p) e -> p t e", p=128))

            def load_w(w_d, nm, pool, eng, fine=False, head_cols=0):
                wt = [pool.tile([128, 4, w_d.shape[1]], bf16, tag="wt",
                                name=f"{nm}{_}") for _ in range(2)]
                if head_cols:
                    eng.dma_start(wt[0][:, 0, 0:head_cols],
                                  w_d[0:128, 0:head_cols])
                    eng.dma_start(wt[0][:, 0, head_cols:],
                                  w_d[0:128, head_cols:])
                for t in range(2):
                    nq = 4 if (fine and t == 0) else 2
                    w_ = 4 // nq
                    for hh in range(nq):
                        if head_cols and t == 0 and hh == 0 and w_ == 1:
                            continue
                        eng.dma_start(
                            wt[t][:, hh * w_:(hh + 1) * w_, :],
                            w_d[t * 512 + hh * w_ * 128:
                                t * 512 + (hh + 1) * w_ * 128, :]
                            .rearrange("(t p) e -> p t e", p=128))
                return wt

            wtv = load_w(wv_d, "wtv", wp, nc.sync, fine=True, head_cols=520)
            wtq = load_w(wq_d, "wtq", wp, nc.sync)
            wtk = load_w(wk_d, "wtk", wkp, nc.gpsimd)
            wot = [wop.tile([128, E], bf16, tag="wo", name=f"wo{_}") for _ in range(8)]
            for t in range(8):
                nc.sync.dma_start(wot[t][:], wo_d[t * 128:(t + 1) * 128, :])

            ebs = {}

            def issue_eb(h):
                # two half-tiles per head: the 4-slot ring lets half 0's DMA
                # start once head h-2 is past kc3 (instead of fully done),
                # and halves the quantum behind which SP-queue neighbours
                # (the vT pair transposes) can get stuck
                eb_lo = ebp.tile([128, KT // 2, L], bf16, tag="eb",
                                 name=f"eb{h}lo")
                eb_hi = ebp.tile([128, KT // 2, L], bf16, tag="eb",
                                 name=f"eb{h}hi")
                for t, ebt in ((0, eb_lo), (1, eb_hi)):
                    nc.sync.dma_start(
                        ebt[:],
                        expb_d[h, t * 512:(t + 1) * 512, :]
                        .rearrange("(kt p) q -> p kt q", p=128))
                ebs[h] = (eb_lo, eb_hi)

            for h in range(3):
                issue_eb(h)

            def xsl(k):
                return xs4[k // 4][:, k % 4, :]

            # ---------------- V projection ----------------
            # Q(0)/K(0) ride in the same psum pool right after V so the
            # first head's S matmuls aren't blocked on a cross-pool handoff.
            segs = [(0, 512), (512, 512), (1024, 16)]
            with tc.tile_pool(name="v_ps", bufs=2, space=PSUM) as vps, \
                 tc.tile_pool(name="v_pro", bufs=1, space=PSUM) as vpro:

                def emit_prologue():
                    for wts, dst, ceng in ((wtq, qT, "v"), (wtk, kTt, "s")):
                        ps = vpro.tile([128, L], f32, tag="pspro", name="ps_pro")
                        for k in range(8):
                            for lh in range(2):
                                nc.tensor.matmul(
                                    ps[:, lh * 512:(lh + 1) * 512],
                                    wts[k // 4][:, k % 4, 0:128],
                                    xsl(k)[:, lh * 512:(lh + 1) * 512],
                                    start=(k == 0), stop=(k == 7))
                        if ceng == "v":
                            nc.vector.tensor_copy(dst[0][:], ps[:])
                        else:
                            nc.scalar.copy(dst[0][:], ps[:])

                for lc in range(8):
                    psv = vps.tile([128, H * 65], f32, tag="psv")
                    for k in range(8):
                        for off, n in segs:
                            nc.tensor.matmul(
                                psv[:, off:off + n],
                                xsl(k)[:, lc * 128:(lc + 1) * 128],
                                wtv[k // 4][:, k % 4, off:off + n],
                                start=(k == 0), stop=(k == 7))
                    nc.scalar.copy(vs[lc][:, 0:520], psv[:, 0:520])
                    nc.vector.tensor_copy(vs[lc][:, 520:], psv[:, 520:])
                    # ones column per head (col 64 of each 65-wide slot)
                    ones_cols = vs[lc][:].rearrange(
                        "p (h c) -> p h c", c=65)[:, :, 64:65]
                    nc.vector.memset(ones_cols, 1.0)
                    if lc == 5:
                        # prologue Q(0)/K(0) here: its copies jump ahead of
                        # the remaining V copies in the ACT/DVE queues
                        emit_prologue()

            # ---------- heads with woven projections ----------
            with tc.tile_pool(name="b_st", bufs=2, space=PSUM) as stp, \
                 tc.tile_pool(name="b_pvn", bufs=1, space=PSUM) as pvnp, \
                 tc.tile_pool(name="b_pj", bufs=2, space=PSUM) as pjp, \
                 tc.tile_pool(name="b_pt", bufs=3) as ptp, \
                 tc.tile_pool(name="b_pm", bufs=3) as pmp, \
                 tc.tile_pool(name="b_nrm", bufs=2) as nrm:

                def emit_proj(ps_q, wts, m, k, g):
                    nc.tensor.matmul(
                        ps_q[:],
                        wts[k // 4][:, k % 4, m * 128:(m + 1) * 128],
                        xsl(k)[:, g * 256:(g + 1) * 256],
                        start=(k == 0), stop=(k == 7))

                def emit_s(h, kc, st):
                    g, hb = h // 2, (h % 2) * 64
                    for qh in range(2):
                        nc.tensor.matmul(
                            st[:, qh * 512:(qh + 1) * 512],
                            kTt[g][hb:hb + 64, kc * 128:(kc + 1) * 128],
                            qT[g][hb:hb + 64, qh * 512:(qh + 1) * 512],
                            start=True, stop=True)

                pending_free = [None]
                hoisted = [None]
                vnat2 = [None]
                # lc=0 out-projection partials woven into head 14 (whose Q/K
                # job queue is empty): [eh][ec<=6 done] psum halves
                psy0 = [None, None]
                # proj jobs are 32 quarter-matmuls (4 psum quarter-groups of
                # 8 contract steps, copy inline after each group), consumed
                # 4 per kc: PE's time between consecutive S(kc) completions
                # stays ~1.07us, just above ACT's 1.04us exp cadence, so the
                # exp stream never waits on a batch of woven matmuls (any
                # ACT slip compounds across heads and stalls PE via the
                # S-tile ring WAR).
                weave_per_kc = [4] * 8
                psy0_weave = [2, 2, 2, 2, 2, 2, 0, 0]
                psy0b_weave = [1, 1, 0, 0, 0, 0, 0, 0]

                def attn_head(h, job):
                    g, hb = h // 2, (h % 2) * 64
                    eb = ebs.pop(h)
                    if job == "psy0":
                        # lc0 out-proj partials, ec 0..5 only: vT[0..5] were
                        # delivered many heads ago, so no unit can ever park
                        # in PE's in-order queue ahead of this head's S
                        # matmuls (which pace the exp stream's finish)
                        psy0[0] = pjp.tile([128, 512], f32, tag="pj",
                                           name="psy0h0")
                        psy0[1] = pjp.tile([128, 512], f32, tag="pj",
                                           name="psy0h1")
                        jobs = [(eh, ec) for ec in range(6)
                                for eh in range(2)]
                    elif job == "psy0b":
                        # head 15: add ec6 (vT[6]'s transpose landed before
                        # this head began)
                        jobs = [(0, 6), (1, 6)]
                    elif job is not None:
                        wts, dst, m = job
                        phq = [None] * 4
                        jobs = [(k, g) for g in range(4) for k in range(8)]
                    else:
                        jobs = []
                    pvn = pvnp.tile([128, KT, 128], f32, tag="pvn")
                    pmq = []
                    ji = 0
                    if hoisted[0] is not None:
                        st_cur = hoisted[0]
                        hoisted[0] = None
                    else:
                        st_cur = stp.tile([128, L], f32, tag="st")
                        emit_s(h, 0, st_cur)
                    for kc in range(KT):
                        # emit the next S ahead of everything else so the ACT
                        # exp stream is never gated on a just-in-time matmul
                        st_next = None
                        if kc + 1 < KT:
                            st_next = stp.tile([128, L], f32, tag="st")
                            emit_s(h, kc + 1, st_next)
                        pt = ptp.tile([128, L], bf16, tag="pt")
                        nc.scalar.activation(pt[:], st_cur[:], Exp,
                                             scale=SCALE)
                        pm = pmp.tile([128, L], bf16, tag="pm")
                        # kc7's multiply on Pool keeps DVE clear for the
                        # final weave copy, which the hoisted S depends on
                        pool_mul = kc in (2, 5) or (kc == 7 and h < H - 1)
                        meng = nc.gpsimd if pool_mul else nc.vector
                        meng.tensor_mul(pm[:], pt[:], eb[kc // 4][:, kc % 4, :])
                        if kc == 1 and h + 3 < H:
                            # eb issued at kc1: its DMA then queues on SP
                            # BEHIND this head's vT transpose (emitted in
                            # pending_free at kc0), so a WAR-blocked eb
                            # can't head-of-line-block the transpose
                            issue_eb(h + 3)
                        if job == "psy0":
                            wkc = psy0_weave
                        elif job == "psy0b":
                            wkc = psy0b_weave
                        else:
                            wkc = weave_per_kc
                        for _ in range(wkc[kc]):
                            if ji < len(jobs):
                                if job in ("psy0", "psy0b"):
                                    eh, ec = jobs[ji]
                                    nc.tensor.matmul(
                                        psy0[eh][:],
                                        vT[ec][:, 0:128],
                                        wot[ec][:, eh * 512:(eh + 1) * 512],
                                        start=(ec == 0), stop=False)
                                else:
                                    k_, g_ = jobs[ji]
                                    if k_ == 0:
                                        phq[g_] = pjp.tile(
                                            [128, 256], f32, tag="pj",
                                            name=f"ph{m}g{g_}")
                                    emit_proj(phq[g_], wts, m, k_, g_)
                                    if k_ == 7:
                                        nc.vector.tensor_copy(
                                            dst[m][:, g_ * 256:(g_ + 1) * 256],
                                            phq[g_][:])
                                ji += 1
                        if len(pmq) == 2:
                            pkc = kc - 2
                            ppm = pmq.pop(0)
                            for qc in range(8):
                                nc.tensor.matmul(
                                    pvn[:, qc, 0:65],
                                    ppm[:, qc * 128:(qc + 1) * 128],
                                    vs[pkc][:, h * 65:(h + 1) * 65],
                                    start=(pkc == 0 and qc % 4 == 0),
                                    stop=False,
                                    skip_group_check=True)
                        if kc == KT - 1 and h + 1 < H:
                            # hoisted next-head S emitted last: the weave's
                            # final qT/kT copy it reads has landed by then
                            nst = stp.tile([128, L], f32, tag="st")
                            emit_s(h + 1, 0, nst)
                            hoisted[0] = nst
                        pmq.append(pm)
                        st_cur = st_next
                        if kc == 0 and pending_free[0] is not None:
                            pending_free[0]()
                            pending_free[0] = None
                    for pkc in (KT - 2, KT - 1):
                        ppm = pmq.pop(0)
                        for qc in range(8):
                            nc.tensor.matmul(
                                pvn[:, qc, 0:65],
                                ppm[:, qc * 128:(qc + 1) * 128],
                                vs[pkc][:, h * 65:(h + 1) * 65],
                                start=False, stop=(pkc == KT - 1),
                                skip_group_check=True)
                    # normalize (values-natural: denominator is per-partition)
                    # into the pair staging tile [q, (kt, ha)]; after the odd
                    # head, one xbar DMA transpose writes vT[g] = [ha, (kt, q)]
                    # (out partition span must equal the 128-wide free blocks
                    # for the hardware xbar lowering, hence pair granularity).
                    r8 = nrm.tile([128, KT, 1], f32, tag="stage")
                    if h % 2 == 0:
                        vnat2[0] = nrm.tile([128, KT, 128], bf16, tag="vtmp",
                                            name=f"vnat{g}")
                    vn = vnat2[0]

                    def free_pv():
                        nc.vector.reciprocal(r8[:], pvn[:, :, 64:65])
                        for qc in range(8):
                            nc.vector.tensor_scalar_mul(
                                vn[:, qc, hb:hb + 64], pvn[:, qc, 0:64],
                                r8[:, qc, :])
                        if h % 2 == 1:
                            nc.sync.dma_start_transpose(
                                vT[g][:].rearrange("p (kt q) -> p kt q", q=128),
                                vn[:].rearrange("p kt a -> p (kt a)"))

                    pending_free[0] = free_pv

                for h in range(H):
                    j = h // 2
                    if h % 2 == 0:
                        job = (wtq, qT, j + 1) if j + 1 < 8 else "psy0"
                    else:
                        job = (wtk, kTt, j + 1) if j + 1 < 8 else "psy0b"
                    attn_head(h, job)
                pending_free[0]()

                # ------------- Phase C: output projection -------------
                # ec=7 (head pair 7, finalized just above) is ordered last
                # per accumulator so its transpose DMA never stalls PE.
                with tc.tile_pool(name="c_y", bufs=2) as yp, \
                     tc.tile_pool(name="c_yq", bufs=4) as yqp:

                    def psy_mm(psy, pcol0, lc, eh, ec, start, stop):
                        nc.tensor.matmul(
                            psy[:, pcol0:pcol0 + 512],
                            vT[ec][:, lc * 128:(lc + 1) * 128],
                            wot[ec][:, eh * 512:(eh + 1) * 512],
                            start=start, stop=stop)

                    def store_y(lc, ysrc, split):
                        # ysrc: list of (psum_tile, col0, eh); copies
                        # Pool/DVE, DMA Pool/SP
                        y = yp.tile([128, E], f32, tag="y")
                        for pst, col0, eh in ysrc:
                            if split:
                                for q in range(2):
                                    cols = slice(eh * 512 + q * 256,
                                                 eh * 512 + (q + 1) * 256)
                                    pcols = slice(col0 + q * 256,
                                                  col0 + (q + 1) * 256)
                                    if (eh + q) % 2:
                                        nc.scalar.copy(y[:, cols],
                                                       pst[:, pcols])
                                        nc.gpsimd.dma_start(
                                            y_d[lc * 128:(lc + 1) * 128, cols],
                                            y[:, cols])
                                    else:
                                        nc.vector.tensor_copy(y[:, cols],
                                                              pst[:, pcols])
                                        nc.sync.dma_start(
                                            y_d[lc * 128:(lc + 1) * 128, cols],
                                            y[:, cols])
                            else:
                                if eh:
                                    nc.scalar.copy(y[:, 512:E],
                                                   pst[:, col0:col0 + 512])
                                    nc.sync.dma_start(
                                        y_d[lc * 128:(lc + 1) * 128, 512:E],
                                        y[:, 512:E])
                                else:
                                    nc.vector.tensor_copy(
                                        y[:, 0:512], pst[:, col0:col0 + 512])
                                    nc.gpsimd.dma_start(
                                        y_d[lc * 128:(lc + 1) * 128, 0:512],
                                        y[:, 0:512])

                    # lc1, lc2 from the freed S pool: ec0..6 first (vT[7]'s
                    # transpose is still in flight)
                    psyA = stp.tile([128, L], f32, tag="st", name="psyA")
                    psyB = stp.tile([128, L], f32, tag="st", name="psyB")
                    for ec in range(7):
                        for eh in range(2):
                            psy_mm(psyA, eh * 512, 1, eh, ec,
                                   start=(ec == 0), stop=False)
                    for ec in range(7):
                        for eh in range(2):
                            psy_mm(psyB, eh * 512, 2, eh, ec,
                                   start=(ec == 0), stop=False)
                    # vT[7] has landed by now
                    for eh in range(2):
                        psy_mm(psy0[eh], 0, 0, eh, 7, start=False, stop=True)
                    store_y(0, [(psy0[0], 0, 0), (psy0[1], 0, 1)], split=False)
                    for eh in range(2):
                        psy_mm(psyA, eh * 512, 1, eh, 7, start=False, stop=True)
                    store_y(1, [(psyA, 0, 0), (psyA, 512, 1)], split=False)
                    for eh in range(2):
                        psy_mm(psyB, eh * 512, 2, eh, 7, start=False, stop=True)
                    store_y(2, [(psyB, 0, 0), (psyB, 512, 1)], split=False)
                    for lc in range(3, 7):
                        # rotate accumulators through a third psum pool
                        # (pvnp is free after the last normalize) so the
                        # 2-slot ring WAR never paces the store pipeline
                        if lc in (4, 6):
                            psy = pvnp.tile([128, KT, 128], f32, tag="pvn",
                                            name=f"psy{lc}")
                            psy = psy[:].rearrange("p a b -> p (a b)")
                        else:
                            psy = stp.tile([128, L], f32, tag="st",
                                           name=f"psy{lc}")
                        for ec in range(8):
                            for eh in range(2):
                                psy_mm(psy, eh * 512, lc, eh, ec,
                                       start=(ec == 0), stop=(ec == 7))
                        store_y(lc, [(psy, 0, 0), (psy, 512, 1)],
                                split=(lc >= 5))
                    # last chunk: accumulate per column-quarter in SEPARATE
                    # psum tiles (per-tile dep tracking would serialize a
                    # shared tile against each quarter's copy) so each
                    # quarter's copy+DMA pipelines behind the next quarter's
                    # matmuls and the final DMA is small and issued early
                    for q in range(4):
                        pool = stp if q < 2 else pjp
                        psq = pool.tile([128, 256], f32,
                                        tag="st" if q < 2 else "pj",
                                        name=f"psq{q}")
                        for ec in range(8):
                            nc.tensor.matmul(
                                psq[:],
                                vT[ec][:, 7 * 128:8 * 128],
                                wot[ec][:, q * 256:(q + 1) * 256],
                                start=(ec == 0), stop=(ec == 7))
                        y = yqp.tile([128, 256], f32, tag="yq", name=f"yq{q}")
                        if q == 3:
                            # final piece split in two so the very last DMA
                            # is small and issued as early as possible
                            for e in range(2):
                                cols = slice(q * 256 + e * 128,
                                             q * 256 + (e + 1) * 128)
                                ecols = slice(e * 128, (e + 1) * 128)
                                if e:
                                    nc.scalar.copy(y[:, ecols], psq[:, ecols])
                                    nc.sync.dma_start(y_d[7 * 128:L, cols],
                                                      y[:, ecols])
                                else:
                                    nc.vector.tensor_copy(y[:, ecols],
                                                          psq[:, ecols])
                                    nc.gpsimd.dma_start(y_d[7 * 128:L, cols],
                                                        y[:, ecols])
                        else:
                            cols = slice(q * 256, (q + 1) * 256)
                            if q % 2:
                                nc.scalar.copy(y[:], psq[:])
                                nc.sync.dma_start(y_d[7 * 128:L, cols], y[:])
                            else:
                                nc.vector.tensor_copy(y[:], psq[:])
                                nc.gpsimd.dma_start(y_d[7 * 128:L, cols], y[:])

    nc.finalize()
    return nc


def _prep_host(inputs):
    import ml_dtypes

    bf = ml_dtypes.bfloat16
    emb = np.asarray(inputs["embeddings"], np.float32)
    mask = np.asarray(inputs["attn_mask"])
    bias = np.asarray(inputs["attn_bias"], np.float32)
    Wqkv = np.asarray(inputs["W_qkv"], np.float32)
    Wout = np.asarray(inputs["W_out"], np.float32)

    Wr = Wqkv.reshape(H, 3 * A, E)
    WqT = np.ascontiguousarray(Wr[:, 0:A, :].reshape(E, E).T.astype(bf))
    WkT = np.ascontiguousarray(Wr[:, A:2 * A, :].reshape(E, E).T.astype(bf))
    Wv_T = Wr[:, 2 * A:3 * A, :].reshape(E, E).T  # [e, (h,a)]
    WvT = np.zeros((E, H * 65), np.float32)
    for h in range(H):
        WvT[:, h * 65:h * 65 + 64] = Wv_T[:, h * 64:(h + 1) * 64]
    WvT = np.ascontiguousarray(WvT.astype(bf))
    WoT = np.ascontiguousarray(Wout.T.astype(bf))

    if mask.dtype != np.bool_:
        mask = mask != 0

    in_maps = []
    for b in range(B):
        # expb^T[h, k, q] = exp(bias[b, h, q, k]) masked to 0, bf16
        expb = np.where(mask[b], 0.0, np.exp(bias[b]))  # [H, q, k]
        expbT = np.ascontiguousarray(expb.transpose(0, 2, 1).astype(bf))
        in_maps.append({
            "xT": np.ascontiguousarray(emb[b].T.astype(bf)),
            "wq": WqT, "wk": WkT, "wv": WvT, "wo": WoT,
            "expb": expbT,
        })
    return in_maps


def _run(inputs, trace=False):
    from concourse.bass_utils import run_bass_kernel_spmd

    if "nc" not in _cache:
        _cache["nc"] = _build_nc()
    nc = _cache["nc"]
    in_maps = _prep_host(inputs)
    res = run_bass_kernel_spmd(nc, in_maps, core_ids=list(range(8)), trace=trace)
    out = np.stack([np.asarray(res.results[c]["y"], np.float32) for c in range(B)], axis=0)
    return out, res


def kernel(**inputs) -> np.ndarray:
    out, _ = _run(inputs, trace=False)
    return out


def kernel_traced(**inputs):
    return _run(inputs, trace=True)
